# revision 34
# baseline (speedup 1.0000x reference)
"""Trainium2 Bass kernel for BinaryMLP.

reference:
    h = relu(x @ sign(W1).T + b1)   # [B, 128], x: [B, 196]
    h = relu(h @ sign(W2).T + b2)   # [B, 128]
    h = relu(h @ sign(W3).T + b3)   # [B, 128]
    y = h @ W4.T + b4               # [B, 10]

Strategy (pure data parallel over 8 cores, 65536 rows each):
  - Host: transpose + bf16-cast the x shard -> xT [196, B_core] so the
    contraction dim lands on SBUF partitions and every DMA is contiguous.
    sign(W) is exact in bf16. b4 is added on host.
  - Device: 512-column batch tiles, processed in weight-paired twos
    (consecutive matmuls share the stationary operand, so LDWEIGHTS is
    hidden), with 2-bank PSUM tensors so each ReLU+bias evacuation covers
    1024 columns. Evacuations alternate between ScalarE and VectorE.
  - Software pipelining: layer stages are emitted with a 2-pair skew
    (L2 of pair i-2, L1 of pair i, L3 of pair i-4 per step) so the
    in-order PE always has independent matmuls to run while evacuations
    complete -> no PE stalls, HAM stays warm.
  - Head (M=10): packed 8 tiles per PSUM bank using 4x column tiling
    (tile_position=(0,32u)) x 2 accumulated zero-masked W4 variants, so
    eight N=512 head matmuls cost ~2 matmul slots and one [128,512] copy
    evacuates all eight. Output stays in the strip layout yTS[128, .]
    (rows 32u+10j+p) -> one [128,512] store per 8 tiles; the host
    unscrambles and adds b4.
"""

import numpy as np
import ml_dtypes

import concourse.bass as bass
from concourse.bass import _add_dep_helper
import concourse.mybir as mybir
import concourse.tile as tile
from concourse import bacc
from concourse.bass_utils import run_bass_kernel_spmd

BF16 = ml_dtypes.bfloat16

B_FULL, D_IN, H, D_OUT = 524288, 196, 128, 10
N_CORES = 8
TB = 512          # batch tile = matmul free dim (one PSUM bank of fp32)
PACK = 8          # tiles per head pack / DMA load / store group
K1A = 128
K1B = D_IN - K1A  # 68


def build_nc(b_core: int, n_cores: int = N_CORES, noload_opt: bool = True):
    """Build the per-core Bass program (SPMD: same program on all cores)."""
    dt = mybir.dt
    nc = bacc.Bacc(
        "TRN2", target_bir_lowering=False, debug=False, num_devices=n_cores
    )

    n_tiles = b_core // TB
    assert b_core % (PACK * TB) == 0
    n_packs = n_tiles // PACK
    n_pairs = n_tiles // 2

    xT = nc.dram_tensor("xT", [D_IN, b_core], dt.bfloat16, kind="ExternalInput").ap()
    w1t = nc.dram_tensor("w1t", [D_IN, H], dt.bfloat16, kind="ExternalInput").ap()
    w2t = nc.dram_tensor("w2t", [H, H], dt.bfloat16, kind="ExternalInput").ap()
    w3t = nc.dram_tensor("w3t", [H, H], dt.bfloat16, kind="ExternalInput").ap()
    w4a = nc.dram_tensor("w4a", [H, 32], dt.bfloat16, kind="ExternalInput").ap()
    w4b = nc.dram_tensor("w4b", [H, 32], dt.bfloat16, kind="ExternalInput").ap()
    b1d = nc.dram_tensor("b1", [H, 1], dt.float32, kind="ExternalInput").ap()
    b2d = nc.dram_tensor("b2", [H, 1], dt.float32, kind="ExternalInput").ap()
    b3d = nc.dram_tensor("b3", [H, 1], dt.float32, kind="ExternalInput").ap()
    # strip-layout output: row 32u+10j+p, cols pk*TB+c  <->  y[(pk*8+4j+u)*TB+c, p]
    yTS = nc.dram_tensor(
        "yTS", [H, n_packs * TB], dt.float32, kind="ExternalOutput"
    ).ap()

    relu = mybir.ActivationFunctionType.Relu

    with tile.TileContext(nc) as tc:
        with (
            tc.tile_pool(name="wpool", bufs=1) as wpool,
            tc.tile_pool(name="xa", bufs=20) as xa_pool,
            tc.tile_pool(name="xb", bufs=20) as xb_pool,
            tc.tile_pool(name="h1p", bufs=12) as h1_pool,
            tc.tile_pool(name="h2p", bufs=12) as h2_pool,
            tc.tile_pool(name="h3p", bufs=16) as h3_pool,
            tc.tile_pool(name="yo", bufs=4) as y_pool,
            tc.tile_pool(name="ps1", bufs=3, space="PSUM") as ps1,
            tc.tile_pool(name="ps2", bufs=2, space="PSUM") as ps2,
            tc.tile_pool(name="ps3", bufs=2, space="PSUM") as ps3,
            tc.tile_pool(name="ps4", bufs=1, space="PSUM") as ps4,
        ):
            # --- load weights/biases once ---
            w1a_sb = wpool.tile([K1A, H], dt.bfloat16)
            nc.gpsimd.dma_start(w1a_sb[:], w1t[0:K1A, :])
            w1b_sb = wpool.tile([K1B, H], dt.bfloat16)
            nc.gpsimd.dma_start(w1b_sb[:], w1t[K1A:D_IN, :])
            w2_sb = wpool.tile([H, H], dt.bfloat16)
            nc.gpsimd.dma_start(w2_sb[:], w2t[:, :])
            w3_sb = wpool.tile([H, H], dt.bfloat16)
            nc.gpsimd.dma_start(w3_sb[:], w3t[:, :])
            w4_sb = [
                wpool.tile([H, 32], dt.bfloat16, tag=f"w4_{j}", name=f"w4_{j}")
                for j in range(2)
            ]
            nc.gpsimd.dma_start(w4_sb[0][:], w4a[:, :])
            nc.gpsimd.dma_start(w4_sb[1][:], w4b[:, :])
            b_sb = []
            for j, bd in enumerate((b1d, b2d, b3d)):
                b = wpool.tile([H, 1], dt.float32, tag=f"b_{j}", name=f"b_{j}")
                nc.gpsimd.dma_start(b[:], bd[:, :])
                b_sb.append(b)

            def relu_evac(use_act, h_out, psum_in, bias_sb):
                if use_act:
                    return nc.scalar.activation(h_out[:], psum_in[:], relu, bias=bias_sb[:])
                else:
                    return nc.vector.tensor_scalar(
                        h_out[:],
                        psum_in[:],
                        bias_sb[:],
                        0.0,
                        mybir.AluOpType.add,
                        mybir.AluOpType.max,
                    )

            noload = []  # matmuls that reuse already-loaded weights
            W = PACK * TB  # columns per load group
            xa_t: dict = {}
            xb_t: dict = {}
            h1_t: dict = {}
            h2_t: dict = {}
            h3_t: dict = {}

            def emit_load(pk):
                # Per-pair quarter tiles: each pair's matmuls wait only on
                # their own 2*TB columns; xb quarter follows its xa quarter.
                c0 = pk * W
                qw = 2 * TB
                for qf in range(4):
                    ch = slice(c0 + qf * qw, c0 + (qf + 1) * qw)
                    xa = xa_pool.tile(
                        [K1A, qw], dt.bfloat16, tag="xa", name=f"xa_{pk}_{qf}"
                    )
                    nc.sync.dma_start(xa[:], xT[0:K1A, ch])
                    xb = xb_pool.tile(
                        [K1B, qw], dt.bfloat16, tag="xb", name=f"xb_{pk}_{qf}"
                    )
                    nc.sync.dma_start(xb[:], xT[K1A:D_IN, ch])
                    xa_t[(pk, qf)], xb_t[(pk, qf)] = xa, xb

            def stage_A(i):  # L1 for pair i: W1a(t0),W1a(t1),W1b(t0),W1b(t1)
                pk, pr = divmod(i, PACK // 2)
                xa, xb = xa_t[(pk, pr)], xb_t[(pk, pr)]
                ps = []
                for q in range(2):
                    t = 2 * i + q
                    o = q * TB
                    p1 = ps1.tile([H, TB], dt.float32, tag="p1", name=f"p1_{t}")
                    mm = nc.tensor.matmul(
                        p1[:], w1a_sb[:], xa[:, o : o + TB], start=True, stop=False
                    )
                    if q == 1 and noload_opt:
                        mm.ins.ldweights = False
                        noload.append(mm.ins)
                    ps.append((t, o, p1))
                for qq, (t, o, p1) in enumerate(ps):
                    mm = nc.tensor.matmul(
                        p1[:], w1b_sb[:], xb[:, o : o + TB], start=False, stop=True
                    )
                    if qq == 1 and noload_opt:
                        mm.ins.ldweights = False
                        noload.append(mm.ins)
                for t, o, p1 in ps:
                    h1 = h1_pool.tile([H, TB], dt.bfloat16, tag="h1", name=f"h1_{t}")
                    relu_evac(t % 2 == 0, h1, p1, b_sb[0])
                    h1_t[t] = h1

            def stage_B(i):  # L2 for pair i
                for q in range(2):
                    t = 2 * i + q
                    h1 = h1_t.pop(t)
                    p2 = ps2.tile([H, TB], dt.float32, tag="p2", name=f"p2_{t}")
                    mm = nc.tensor.matmul(p2[:], w2_sb[:], h1[:], start=True, stop=True)
                    if q == 1 and noload_opt:
                        mm.ins.ldweights = False
                        noload.append(mm.ins)
                    h2 = h2_pool.tile([H, TB], dt.bfloat16, tag="h2", name=f"h2_{t}")
                    relu_evac(t % 2 == 1, h2, p2, b_sb[1])
                    h2_t[t] = h2

            def stage_C(i):  # L3 for pair i
                for q in range(2):
                    t = 2 * i + q
                    h2 = h2_t.pop(t)
                    p3 = ps3.tile([H, TB], dt.float32, tag="p3", name=f"p3_{t}")
                    mm = nc.tensor.matmul(p3[:], w3_sb[:], h2[:], start=True, stop=True)
                    if q == 1 and noload_opt:
                        mm.ins.ldweights = False
                        noload.append(mm.ins)
                    h3 = h3_pool.tile([H, TB], dt.bfloat16, tag="h3", name=f"h3_{t}")
                    e3 = relu_evac(t % 2 == 0, h3, p3, b_sb[2])
                    h3_t[t] = (h3, e3)

            p4_t: dict = {}
            ysb_t: dict = {}

            def stage_Hj(pk, j):
                # head burst: variant j covers tiles 4j+u (u=0..3) of the
                # pack, i.e. pairs 4pk+2j, 4pk+2j+1 -- both freshly evacuated,
                # so the 4 matmuls are simultaneously ready and stay
                # contiguous -> concurrent via 4x column tiling.
                if j == 0:
                    p4_t[pk] = ps4.tile([H, TB], dt.float32, tag="p4", name=f"p4_{pk}")
                p4 = p4_t[pk]
                tiles = [8 * pk + 4 * j + u for u in range(4)]
                hs = [h3_t[t] for t in tiles]
                # Explicit per-col-group weight loads, then non-self-loading
                # matmuls: interleaved LDW+MM would serialize (a col-group LDW
                # conflicts with every in-flight MM on row groups). All 8
                # instructions become ready at the same event (both L3 evacs
                # done), so the burst schedules contiguously and nothing can
                # clobber the loaded weights in between.
                ldws = []
                for u in range(4):
                    ldw = nc.tensor.ldweights(
                        w4_sb[j][:], tile_position=(0, 32 * u)
                    )
                    for _, e3 in hs:
                        _add_dep_helper(ldw.ins, e3.ins, True, "head ldw after e3")
                    ldws.append(ldw)
                for u in range(4):
                    mm = nc.tensor.matmul(
                        p4[32 * u : 32 * u + 32, :],
                        w4_sb[j][:],
                        hs[u][0][:],
                        start=(j == 0),
                        stop=(j == 1),
                        tile_position=(0, 32 * u),
                        skip_group_check=True,
                    )
                    mm.ins.ldweights = False
                    _add_dep_helper(mm.ins, ldws[u].ins, False, "head mm after ldw")
                for t in tiles:
                    h3_t.pop(t)
                if j == 1:
                    p4 = p4_t.pop(pk)
                    ysb = y_pool.tile([H, TB], dt.float32, tag="ysb", name=f"ysb_{pk}")
                    nc.scalar.copy(ysb[:], p4[:])
                    # per-pack stores on GpSimd (SWDGE): small bursts that
                    # don't starve the next pack's loads, and their waits
                    # never block load triggers on the Sync sequencer
                    nc.gpsimd.dma_start(yTS[:, pk * TB : (pk + 1) * TB], ysb[:])

            # --- software-pipelined emission ---
            emit_load(0)
            for step in range(n_pairs + 11):
                # staggered prefetch: packs 1,2 during steps 0,1, then
                # pack (step+10)//4 at steps 2, 6, 10, ...
                if step in (0, 1) and step + 1 < n_packs:
                    emit_load(step + 1)
                if step % 4 == 0 and (step + 12) // 4 < n_packs:
                    emit_load((step + 12) // 4)
                ib = step - 2
                if 0 <= ib < n_pairs:
                    stage_B(ib)
                if step < n_pairs:
                    stage_A(step)
                ic = step - 4
                if 0 <= ic < n_pairs:
                    stage_C(ic)
                    if ic % 4 == 1:
                        stage_Hj(ic // 4, 0)
                    elif ic % 4 == 3:
                        stage_Hj(ic // 4, 1)

    nc.compile()
    if noload_opt:
        try:
            _verify_noload_safety(nc, noload)
        except AssertionError:
            # schedule changed in a way that makes weight reuse unsafe;
            # rebuild without the optimization (correctness first)
            return build_nc(b_core, n_cores, noload_opt=False)
    return nc


def _weights_key(inst, idx):
    ap = inst.ins[idx]
    s = str(ap)
    return s


def _verify_noload_safety(nc, noload):
    """The schedule is static: verify no other weight load lands between a
    ldweights=False matmul and the instruction that loaded its weights."""
    import concourse.mybir as mybir

    noload_ids = {id(i) for i in noload}
    cur = None  # weights key currently in the PE array (full-array loads)
    checked = 0
    insts = []
    for blk in nc.m.functions[0].blocks:
        insts.extend(blk.instructions)
    for inst in insts:
        if inst.engine != mybir.EngineType.PE:
            continue
        kind = type(inst).__name__
        if kind == "InstLdweights":
            tp = getattr(inst, "tile_position", None)
            if not tp or tuple(tp) == (0, 0):
                cur = _weights_key(inst, 0)
            else:
                cur = ("coltile", None)  # partial col-group load
        elif kind == "InstMatmult":
            if id(inst) in noload_ids:
                want = _weights_key(inst, 1)
                assert cur == want, (
                    f"noload matmul {inst.name} expects weights {want}, array has {cur}"
                )
                checked += 1
            elif getattr(inst, "ldweights", None) is False:
                pass  # head matmul: guarded by its own explicit ldw deps
            else:
                tp = getattr(inst, "tile_position", None)
                if not tp or tuple(tp) == (0, 0):
                    cur = _weights_key(inst, 1)
                else:
                    cur = ("coltile", None)
    assert checked == len(noload), (checked, len(noload))


def _prep_core_inputs(x_shard: np.ndarray, weights: dict) -> dict:
    xT = np.ascontiguousarray(x_shard.T).astype(BF16)
    return {"xT": xT, **weights}


def _prep_weights(W1, b1, W2, b2, W3, b3, W4) -> dict:
    w4a = np.zeros((32, H), np.float32)
    w4a[0:D_OUT] = W4
    w4b = np.zeros((32, H), np.float32)
    w4b[D_OUT : 2 * D_OUT] = W4
    return {
        "w1t": np.ascontiguousarray(np.sign(W1).T).astype(BF16),
        "w2t": np.ascontiguousarray(np.sign(W2).T).astype(BF16),
        "w3t": np.ascontiguousarray(np.sign(W3).T).astype(BF16),
        "w4a": np.ascontiguousarray(w4a.T).astype(BF16),
        "w4b": np.ascontiguousarray(w4b.T).astype(BF16),
        "b1": b1.reshape(H, 1).astype(np.float32),
        "b2": b2.reshape(H, 1).astype(np.float32),
        "b3": b3.reshape(H, 1).astype(np.float32),
    }


def _unscramble(yTS: np.ndarray, b_core: int) -> np.ndarray:
    """yTS [128, n_packs*TB] strip layout -> y_core [b_core, 10]."""
    n_packs = b_core // (PACK * TB)
    # yTS[32u+10j+p, pk*TB+c] = y[(pk*8+4j+u)*TB + c, p]
    v = yTS.reshape(4, 32, n_packs, TB)[:, :20]  # [u, 10j+p, pk, c]
    v = v.reshape(4, 2, 10, n_packs, TB)  # [u, j, p, pk, c]
    # -> y[pk, j, u, c, p]
    y = v.transpose(3, 1, 0, 4, 2).reshape(b_core, D_OUT)
    return y


_NC_CACHE: dict = {}


def run(x, W1, b1, W2, b2, W3, b3, W4, b4, trace=False, trace_kwargs=None):
    """Run the SPMD kernel on 8 cores; returns (y, BassKernelResults)."""
    x = np.asarray(x, dtype=np.float32)
    b_total = x.shape[0]
    assert b_total % N_CORES == 0
    b_core = b_total // N_CORES

    key = b_core
    if key not in _NC_CACHE:
        _NC_CACHE[key] = build_nc(b_core)
    nc = _NC_CACHE[key]

    weights = _prep_weights(
        np.asarray(W1), np.asarray(b1), np.asarray(W2), np.asarray(b2),
        np.asarray(W3), np.asarray(b3), np.asarray(W4),
    )
    in_maps = [
        _prep_core_inputs(x[c * b_core : (c + 1) * b_core], weights)
        for c in range(N_CORES)
    ]
    res = run_bass_kernel_spmd(
        nc,
        in_maps,
        list(range(N_CORES)),
        trace=trace,
        **(trace_kwargs or {}),
    )
    b4f = np.asarray(b4, dtype=np.float32)
    y = np.empty((b_total, D_OUT), dtype=np.float32)
    for c in range(N_CORES):
        y[c * b_core : (c + 1) * b_core] = _unscramble(res.results[c]["yTS"], b_core)
    y += b4f
    return y, res


def kernel(x, W1, b1, W2, b2, W3, b3, W4, b4):
    y, _ = run(x, W1, b1, W2, b2, W3, b3, W4, b4)
    return y


# revision 35
# speedup vs baseline: 1.0175x; 1.0175x over previous
"""Trainium2 Bass kernel for BinaryMLP.

reference:
    h = relu(x @ sign(W1).T + b1)   # [B, 128], x: [B, 196]
    h = relu(h @ sign(W2).T + b2)   # [B, 128]
    h = relu(h @ sign(W3).T + b3)   # [B, 128]
    y = h @ W4.T + b4               # [B, 10]

Strategy (pure data parallel over 8 cores, 65536 rows each):
  - Host: transpose + bf16-cast the x shard -> xT [196, B_core] so the
    contraction dim lands on SBUF partitions and every DMA is contiguous.
    sign(W) is exact in bf16. b4 is added on host.
  - Device: 512-column batch tiles, processed in weight-paired twos
    (consecutive matmuls share the stationary operand, so LDWEIGHTS is
    hidden), with 2-bank PSUM tensors so each ReLU+bias evacuation covers
    1024 columns. Evacuations alternate between ScalarE and VectorE.
  - Software pipelining: layer stages are emitted with a 2-pair skew
    (L2 of pair i-2, L1 of pair i, L3 of pair i-4 per step) so the
    in-order PE always has independent matmuls to run while evacuations
    complete -> no PE stalls, HAM stays warm.
  - Head (M=10): packed 8 tiles per PSUM bank using 4x column tiling
    (tile_position=(0,32u)) x 2 accumulated zero-masked W4 variants, so
    eight N=512 head matmuls cost ~2 matmul slots and one [128,512] copy
    evacuates all eight. Output stays in the strip layout yTS[128, .]
    (rows 32u+10j+p) -> one [128,512] store per 8 tiles; the host
    unscrambles and adds b4.
"""

import numpy as np
import ml_dtypes

import concourse.bass as bass
from concourse.bass import _add_dep_helper
import concourse.mybir as mybir
import concourse.tile as tile
from concourse import bacc
from concourse.bass_utils import run_bass_kernel_spmd

BF16 = ml_dtypes.bfloat16

B_FULL, D_IN, H, D_OUT = 524288, 196, 128, 10
N_CORES = 8
TB = 512          # batch tile = matmul free dim (one PSUM bank of fp32)
PACK = 8          # tiles per head pack / DMA load / store group
K1A = 128
K1B = D_IN - K1A  # 68


def build_nc(b_core: int, n_cores: int = N_CORES, noload_opt: bool = True):
    """Build the per-core Bass program (SPMD: same program on all cores)."""
    dt = mybir.dt
    nc = bacc.Bacc(
        "TRN2", target_bir_lowering=False, debug=False, num_devices=n_cores
    )

    n_tiles = b_core // TB
    assert b_core % (PACK * TB) == 0
    n_packs = n_tiles // PACK
    n_pairs = n_tiles // 2

    xT = nc.dram_tensor("xT", [D_IN, b_core], dt.bfloat16, kind="ExternalInput").ap()
    w1t = nc.dram_tensor("w1t", [D_IN, H], dt.bfloat16, kind="ExternalInput").ap()
    w2t = nc.dram_tensor("w2t", [H, H], dt.bfloat16, kind="ExternalInput").ap()
    w3t = nc.dram_tensor("w3t", [H, H], dt.bfloat16, kind="ExternalInput").ap()
    w4a = nc.dram_tensor("w4a", [H, 32], dt.bfloat16, kind="ExternalInput").ap()
    w4b = nc.dram_tensor("w4b", [H, 32], dt.bfloat16, kind="ExternalInput").ap()
    b1d = nc.dram_tensor("b1", [H, 1], dt.float32, kind="ExternalInput").ap()
    b2d = nc.dram_tensor("b2", [H, 1], dt.float32, kind="ExternalInput").ap()
    b3d = nc.dram_tensor("b3", [H, 1], dt.float32, kind="ExternalInput").ap()
    # strip-layout output: row 32u+10j+p, cols pk*TB+c  <->  y[(pk*8+4j+u)*TB+c, p]
    yTS = nc.dram_tensor(
        "yTS", [H, n_packs * TB], dt.float32, kind="ExternalOutput"
    ).ap()

    relu = mybir.ActivationFunctionType.Relu

    with tile.TileContext(nc) as tc:
        with (
            tc.tile_pool(name="wpool", bufs=1) as wpool,
            tc.tile_pool(name="xa", bufs=16) as xa_pool,
            tc.tile_pool(name="xb", bufs=16) as xb_pool,
            tc.tile_pool(name="h1p", bufs=9) as h1_pool,
            tc.tile_pool(name="h2p", bufs=9) as h2_pool,
            tc.tile_pool(name="h3p", bufs=14) as h3_pool,
            tc.tile_pool(name="yo", bufs=3) as y_pool,
            tc.tile_pool(name="ps1", bufs=3, space="PSUM") as ps1,
            tc.tile_pool(name="ps2", bufs=2, space="PSUM") as ps2,
            tc.tile_pool(name="ps3", bufs=2, space="PSUM") as ps3,
            tc.tile_pool(name="ps4", bufs=1, space="PSUM") as ps4,
        ):
            # --- load weights/biases once ---
            w1a_sb = wpool.tile([K1A, H], dt.bfloat16)
            nc.gpsimd.dma_start(w1a_sb[:], w1t[0:K1A, :])
            w1b_sb = wpool.tile([K1B, H], dt.bfloat16)
            nc.gpsimd.dma_start(w1b_sb[:], w1t[K1A:D_IN, :])
            w2_sb = wpool.tile([H, H], dt.bfloat16)
            nc.gpsimd.dma_start(w2_sb[:], w2t[:, :])
            w3_sb = wpool.tile([H, H], dt.bfloat16)
            nc.gpsimd.dma_start(w3_sb[:], w3t[:, :])
            w4_sb = [
                wpool.tile([H, 32], dt.bfloat16, tag=f"w4_{j}", name=f"w4_{j}")
                for j in range(2)
            ]
            nc.gpsimd.dma_start(w4_sb[0][:], w4a[:, :])
            nc.gpsimd.dma_start(w4_sb[1][:], w4b[:, :])
            b_sb = []
            for j, bd in enumerate((b1d, b2d, b3d)):
                b = wpool.tile([H, 1], dt.float32, tag=f"b_{j}", name=f"b_{j}")
                nc.gpsimd.dma_start(b[:], bd[:, :])
                b_sb.append(b)

            def relu_evac(use_act, h_out, psum_in, bias_sb):
                if use_act:
                    return nc.scalar.activation(h_out[:], psum_in[:], relu, bias=bias_sb[:])
                else:
                    return nc.vector.tensor_scalar(
                        h_out[:],
                        psum_in[:],
                        bias_sb[:],
                        0.0,
                        mybir.AluOpType.add,
                        mybir.AluOpType.max,
                    )

            noload = []  # matmuls that reuse already-loaded weights
            W = PACK * TB  # columns per load group
            xa_t: dict = {}
            xb_t: dict = {}
            h1_t: dict = {}
            h2_t: dict = {}
            h3_t: dict = {}

            def emit_load(pk):
                # Per-pair quarter tiles: each pair's matmuls wait only on
                # their own 2*TB columns; xb quarter follows its xa quarter.
                c0 = pk * W
                qw = 2 * TB
                for qf in range(4):
                    ch = slice(c0 + qf * qw, c0 + (qf + 1) * qw)
                    xa = xa_pool.tile(
                        [K1A, qw], dt.bfloat16, tag="xa", name=f"xa_{pk}_{qf}"
                    )
                    nc.sync.dma_start(xa[:], xT[0:K1A, ch])
                    xb = xb_pool.tile(
                        [K1B, qw], dt.bfloat16, tag="xb", name=f"xb_{pk}_{qf}"
                    )
                    nc.sync.dma_start(xb[:], xT[K1A:D_IN, ch])
                    xa_t[(pk, qf)], xb_t[(pk, qf)] = xa, xb

            def stage_A(i):  # L1 for pair i: W1a(t0),W1a(t1),W1b(t0),W1b(t1)
                pk, pr = divmod(i, PACK // 2)
                xa, xb = xa_t[(pk, pr)], xb_t[(pk, pr)]
                ps = []
                for q in range(2):
                    t = 2 * i + q
                    o = q * TB
                    p1 = ps1.tile([H, TB], dt.float32, tag="p1", name=f"p1_{t}")
                    mm = nc.tensor.matmul(
                        p1[:], w1a_sb[:], xa[:, o : o + TB], start=True, stop=False
                    )
                    if q == 1 and noload_opt:
                        mm.ins.ldweights = False
                        noload.append(mm.ins)
                    ps.append((t, o, p1))
                for qq, (t, o, p1) in enumerate(ps):
                    mm = nc.tensor.matmul(
                        p1[:], w1b_sb[:], xb[:, o : o + TB], start=False, stop=True
                    )
                    if qq == 1 and noload_opt:
                        mm.ins.ldweights = False
                        noload.append(mm.ins)
                for t, o, p1 in ps:
                    h1 = h1_pool.tile([H, TB], dt.bfloat16, tag="h1", name=f"h1_{t}")
                    relu_evac(t % 2 == 0, h1, p1, b_sb[0])
                    h1_t[t] = h1

            def stage_B(i):  # L2 for pair i
                for q in range(2):
                    t = 2 * i + q
                    h1 = h1_t.pop(t)
                    p2 = ps2.tile([H, TB], dt.float32, tag="p2", name=f"p2_{t}")
                    mm = nc.tensor.matmul(p2[:], w2_sb[:], h1[:], start=True, stop=True)
                    if q == 1 and noload_opt:
                        mm.ins.ldweights = False
                        noload.append(mm.ins)
                    h2 = h2_pool.tile([H, TB], dt.bfloat16, tag="h2", name=f"h2_{t}")
                    relu_evac(t % 2 == 1, h2, p2, b_sb[1])
                    h2_t[t] = h2

            def stage_C(i):  # L3 for pair i
                for q in range(2):
                    t = 2 * i + q
                    h2 = h2_t.pop(t)
                    p3 = ps3.tile([H, TB], dt.float32, tag="p3", name=f"p3_{t}")
                    mm = nc.tensor.matmul(p3[:], w3_sb[:], h2[:], start=True, stop=True)
                    if q == 1 and noload_opt:
                        mm.ins.ldweights = False
                        noload.append(mm.ins)
                    h3 = h3_pool.tile([H, TB], dt.bfloat16, tag="h3", name=f"h3_{t}")
                    e3 = relu_evac(t % 2 == 0, h3, p3, b_sb[2])
                    h3_t[t] = (h3, e3)

            p4_t: dict = {}
            ysb_t: dict = {}

            def stage_Hj(pk, j):
                # head burst: variant j covers tiles 4j+u (u=0..3) of the
                # pack, i.e. pairs 4pk+2j, 4pk+2j+1 -- both freshly evacuated,
                # so the 4 matmuls are simultaneously ready and stay
                # contiguous -> concurrent via 4x column tiling.
                if j == 0:
                    p4_t[pk] = ps4.tile([H, TB], dt.float32, tag="p4", name=f"p4_{pk}")
                p4 = p4_t[pk]
                tiles = [8 * pk + 4 * j + u for u in range(4)]
                hs = [h3_t[t] for t in tiles]
                # Explicit per-col-group weight loads, then non-self-loading
                # matmuls: interleaved LDW+MM would serialize (a col-group LDW
                # conflicts with every in-flight MM on row groups). All 8
                # instructions become ready at the same event (both L3 evacs
                # done), so the burst schedules contiguously and nothing can
                # clobber the loaded weights in between.
                ldws = []
                for u in range(4):
                    ldw = nc.tensor.ldweights(
                        w4_sb[j][:], tile_position=(0, 32 * u)
                    )
                    for _, e3 in hs:
                        _add_dep_helper(ldw.ins, e3.ins, True, "head ldw after e3")
                    ldws.append(ldw)
                for u in range(4):
                    mm = nc.tensor.matmul(
                        p4[32 * u : 32 * u + 32, :],
                        w4_sb[j][:],
                        hs[u][0][:],
                        start=(j == 0),
                        stop=(j == 1),
                        tile_position=(0, 32 * u),
                        skip_group_check=True,
                    )
                    mm.ins.ldweights = False
                    _add_dep_helper(mm.ins, ldws[u].ins, False, "head mm after ldw")
                for t in tiles:
                    h3_t.pop(t)
                if j == 1:
                    p4 = p4_t.pop(pk)
                    ysb = y_pool.tile([H, TB], dt.float32, tag="ysb", name=f"ysb_{pk}")
                    nc.scalar.copy(ysb[:], p4[:])
                    # per-pack stores on GpSimd (SWDGE): small bursts that
                    # don't starve the next pack's loads, and their waits
                    # never block load triggers on the Sync sequencer
                    nc.gpsimd.dma_start(yTS[:, pk * TB : (pk + 1) * TB], ysb[:])

            # --- software-pipelined emission ---
            emit_load(0)
            for step in range(n_pairs + 11):
                # staggered prefetch: packs 1,2 during steps 0,1, then
                # pack (step+10)//4 at steps 2, 6, 10, ...
                if step in (0, 1) and step + 1 < n_packs:
                    emit_load(step + 1)
                if step % 4 == 0 and (step + 12) // 4 < n_packs:
                    emit_load((step + 12) // 4)
                ib = step - 2
                if 0 <= ib < n_pairs:
                    stage_B(ib)
                if step < n_pairs:
                    stage_A(step)
                ic = step - 4
                if 0 <= ic < n_pairs:
                    stage_C(ic)
                    if ic % 4 == 1:
                        stage_Hj(ic // 4, 0)
                    elif ic % 4 == 3:
                        stage_Hj(ic // 4, 1)

    nc.compile()
    if noload_opt:
        try:
            _verify_noload_safety(nc, noload)
        except AssertionError:
            # schedule changed in a way that makes weight reuse unsafe;
            # rebuild without the optimization (correctness first)
            return build_nc(b_core, n_cores, noload_opt=False)
    return nc


def _weights_key(inst, idx):
    ap = inst.ins[idx]
    s = str(ap)
    return s


def _verify_noload_safety(nc, noload):
    """The schedule is static: verify no other weight load lands between a
    ldweights=False matmul and the instruction that loaded its weights."""
    import concourse.mybir as mybir

    noload_ids = {id(i) for i in noload}
    cur = None  # weights key currently in the PE array (full-array loads)
    checked = 0
    insts = []
    for blk in nc.m.functions[0].blocks:
        insts.extend(blk.instructions)
    for inst in insts:
        if inst.engine != mybir.EngineType.PE:
            continue
        kind = type(inst).__name__
        if kind == "InstLdweights":
            tp = getattr(inst, "tile_position", None)
            if not tp or tuple(tp) == (0, 0):
                cur = _weights_key(inst, 0)
            else:
                cur = ("coltile", None)  # partial col-group load
        elif kind == "InstMatmult":
            if id(inst) in noload_ids:
                want = _weights_key(inst, 1)
                assert cur == want, (
                    f"noload matmul {inst.name} expects weights {want}, array has {cur}"
                )
                checked += 1
            elif getattr(inst, "ldweights", None) is False:
                pass  # head matmul: guarded by its own explicit ldw deps
            else:
                tp = getattr(inst, "tile_position", None)
                if not tp or tuple(tp) == (0, 0):
                    cur = _weights_key(inst, 1)
                else:
                    cur = ("coltile", None)
    assert checked == len(noload), (checked, len(noload))


def _prep_core_inputs(x_shard: np.ndarray, weights: dict) -> dict:
    xT = np.ascontiguousarray(x_shard.T).astype(BF16)
    return {"xT": xT, **weights}


def _prep_weights(W1, b1, W2, b2, W3, b3, W4) -> dict:
    w4a = np.zeros((32, H), np.float32)
    w4a[0:D_OUT] = W4
    w4b = np.zeros((32, H), np.float32)
    w4b[D_OUT : 2 * D_OUT] = W4
    return {
        "w1t": np.ascontiguousarray(np.sign(W1).T).astype(BF16),
        "w2t": np.ascontiguousarray(np.sign(W2).T).astype(BF16),
        "w3t": np.ascontiguousarray(np.sign(W3).T).astype(BF16),
        "w4a": np.ascontiguousarray(w4a.T).astype(BF16),
        "w4b": np.ascontiguousarray(w4b.T).astype(BF16),
        "b1": b1.reshape(H, 1).astype(np.float32),
        "b2": b2.reshape(H, 1).astype(np.float32),
        "b3": b3.reshape(H, 1).astype(np.float32),
    }


def _unscramble(yTS: np.ndarray, b_core: int) -> np.ndarray:
    """yTS [128, n_packs*TB] strip layout -> y_core [b_core, 10]."""
    n_packs = b_core // (PACK * TB)
    # yTS[32u+10j+p, pk*TB+c] = y[(pk*8+4j+u)*TB + c, p]
    v = yTS.reshape(4, 32, n_packs, TB)[:, :20]  # [u, 10j+p, pk, c]
    v = v.reshape(4, 2, 10, n_packs, TB)  # [u, j, p, pk, c]
    # -> y[pk, j, u, c, p]
    y = v.transpose(3, 1, 0, 4, 2).reshape(b_core, D_OUT)
    return y


_NC_CACHE: dict = {}


def run(x, W1, b1, W2, b2, W3, b3, W4, b4, trace=False, trace_kwargs=None):
    """Run the SPMD kernel on 8 cores; returns (y, BassKernelResults)."""
    x = np.asarray(x, dtype=np.float32)
    b_total = x.shape[0]
    assert b_total % N_CORES == 0
    b_core = b_total // N_CORES

    key = b_core
    if key not in _NC_CACHE:
        _NC_CACHE[key] = build_nc(b_core)
    nc = _NC_CACHE[key]

    weights = _prep_weights(
        np.asarray(W1), np.asarray(b1), np.asarray(W2), np.asarray(b2),
        np.asarray(W3), np.asarray(b3), np.asarray(W4),
    )
    in_maps = [
        _prep_core_inputs(x[c * b_core : (c + 1) * b_core], weights)
        for c in range(N_CORES)
    ]
    res = run_bass_kernel_spmd(
        nc,
        in_maps,
        list(range(N_CORES)),
        trace=trace,
        **(trace_kwargs or {}),
    )
    b4f = np.asarray(b4, dtype=np.float32)
    y = np.empty((b_total, D_OUT), dtype=np.float32)
    for c in range(N_CORES):
        y[c * b_core : (c + 1) * b_core] = _unscramble(res.results[c]["yTS"], b_core)
    y += b4f
    return y, res


def kernel(x, W1, b1, W2, b2, W3, b3, W4, b4):
    y, _ = run(x, W1, b1, W2, b2, W3, b3, W4, b4)
    return y


# revision 36
# speedup vs baseline: 1.0280x; 1.0103x over previous
"""Trainium2 Bass kernel for BinaryMLP.

reference:
    h = relu(x @ sign(W1).T + b1)   # [B, 128], x: [B, 196]
    h = relu(h @ sign(W2).T + b2)   # [B, 128]
    h = relu(h @ sign(W3).T + b3)   # [B, 128]
    y = h @ W4.T + b4               # [B, 10]

Strategy (pure data parallel over 8 cores, 65536 rows each):
  - Host: transpose + bf16-cast the x shard -> xT [196, B_core] so the
    contraction dim lands on SBUF partitions and every DMA is contiguous.
    sign(W) is exact in bf16. b4 is added on host.
  - Device: 512-column batch tiles, processed in weight-paired twos
    (consecutive matmuls share the stationary operand, so LDWEIGHTS is
    hidden), with 2-bank PSUM tensors so each ReLU+bias evacuation covers
    1024 columns. Evacuations alternate between ScalarE and VectorE.
  - Software pipelining: layer stages are emitted with a 2-pair skew
    (L2 of pair i-2, L1 of pair i, L3 of pair i-4 per step) so the
    in-order PE always has independent matmuls to run while evacuations
    complete -> no PE stalls, HAM stays warm.
  - Head (M=10): packed 8 tiles per PSUM bank using 4x column tiling
    (tile_position=(0,32u)) x 2 accumulated zero-masked W4 variants, so
    eight N=512 head matmuls cost ~2 matmul slots and one [128,512] copy
    evacuates all eight. Output stays in the strip layout yTS[128, .]
    (rows 32u+10j+p) -> one [128,512] store per 8 tiles; the host
    unscrambles and adds b4.
"""

import numpy as np
import ml_dtypes

import concourse.bass as bass
from concourse.bass import _add_dep_helper
import concourse.mybir as mybir
import concourse.tile as tile
from concourse import bacc
from concourse.bass_utils import run_bass_kernel_spmd

BF16 = ml_dtypes.bfloat16

B_FULL, D_IN, H, D_OUT = 524288, 196, 128, 10
N_CORES = 8
TB = 512          # batch tile = matmul free dim (one PSUM bank of fp32)
PACK = 8          # tiles per head pack / DMA load / store group
K1A = 128
K1B = D_IN - K1A  # 68


def build_nc(b_core: int, n_cores: int = N_CORES, noload_opt: bool = True):
    """Build the per-core Bass program (SPMD: same program on all cores)."""
    dt = mybir.dt
    nc = bacc.Bacc(
        "TRN2", target_bir_lowering=False, debug=False, num_devices=n_cores
    )

    n_tiles = b_core // TB
    assert b_core % (PACK * TB) == 0
    n_packs = n_tiles // PACK
    n_pairs = n_tiles // 2

    xT = nc.dram_tensor("xT", [D_IN, b_core], dt.bfloat16, kind="ExternalInput").ap()
    w1t = nc.dram_tensor("w1t", [D_IN, H], dt.bfloat16, kind="ExternalInput").ap()
    w2t = nc.dram_tensor("w2t", [H, H], dt.bfloat16, kind="ExternalInput").ap()
    w3t = nc.dram_tensor("w3t", [H, H], dt.bfloat16, kind="ExternalInput").ap()
    w4a = nc.dram_tensor("w4a", [H, 32], dt.bfloat16, kind="ExternalInput").ap()
    w4b = nc.dram_tensor("w4b", [H, 32], dt.bfloat16, kind="ExternalInput").ap()
    b1d = nc.dram_tensor("b1", [H, 1], dt.float32, kind="ExternalInput").ap()
    b2d = nc.dram_tensor("b2", [H, 1], dt.float32, kind="ExternalInput").ap()
    b3d = nc.dram_tensor("b3", [H, 1], dt.float32, kind="ExternalInput").ap()
    # strip-layout output: row 32u+10j+p, cols pk*TB+c  <->  y[(pk*8+4j+u)*TB+c, p]
    yTS = nc.dram_tensor(
        "yTS", [H, n_packs * TB], dt.float32, kind="ExternalOutput"
    ).ap()

    relu = mybir.ActivationFunctionType.Relu

    with tile.TileContext(nc) as tc:
        with (
            tc.tile_pool(name="wpool", bufs=1) as wpool,
            tc.tile_pool(name="xa", bufs=16) as xa_pool,
            tc.tile_pool(name="xb", bufs=16) as xb_pool,
            tc.tile_pool(name="h1p", bufs=9) as h1_pool,
            tc.tile_pool(name="h2p", bufs=9) as h2_pool,
            tc.tile_pool(name="h3p", bufs=14) as h3_pool,
            tc.tile_pool(name="yo", bufs=3) as y_pool,
            tc.tile_pool(name="ps1", bufs=3, space="PSUM") as ps1,
            tc.tile_pool(name="ps2", bufs=2, space="PSUM") as ps2,
            tc.tile_pool(name="ps3", bufs=2, space="PSUM") as ps3,
            tc.tile_pool(name="ps4", bufs=1, space="PSUM") as ps4,
        ):
            # --- load weights/biases once ---
            w1a_sb = wpool.tile([K1A, H], dt.bfloat16)
            nc.gpsimd.dma_start(w1a_sb[:], w1t[0:K1A, :])
            w1b_sb = wpool.tile([K1B, H], dt.bfloat16)
            nc.gpsimd.dma_start(w1b_sb[:], w1t[K1A:D_IN, :])
            w2_sb = wpool.tile([H, H], dt.bfloat16)
            nc.gpsimd.dma_start(w2_sb[:], w2t[:, :])
            w3_sb = wpool.tile([H, H], dt.bfloat16)
            nc.gpsimd.dma_start(w3_sb[:], w3t[:, :])
            w4_sb = [
                wpool.tile([H, 32], dt.bfloat16, tag=f"w4_{j}", name=f"w4_{j}")
                for j in range(2)
            ]
            nc.gpsimd.dma_start(w4_sb[0][:], w4a[:, :])
            nc.gpsimd.dma_start(w4_sb[1][:], w4b[:, :])
            b_sb = []
            for j, bd in enumerate((b1d, b2d, b3d)):
                b = wpool.tile([H, 1], dt.float32, tag=f"b_{j}", name=f"b_{j}")
                nc.gpsimd.dma_start(b[:], bd[:, :])
                b_sb.append(b)

            def relu_evac(use_act, h_out, psum_in, bias_sb):
                if use_act:
                    return nc.scalar.activation(h_out[:], psum_in[:], relu, bias=bias_sb[:])
                else:
                    return nc.vector.tensor_scalar(
                        h_out[:],
                        psum_in[:],
                        bias_sb[:],
                        0.0,
                        mybir.AluOpType.add,
                        mybir.AluOpType.max,
                    )

            noload = []  # matmuls that reuse already-loaded weights
            W = PACK * TB  # columns per load group
            xa_t: dict = {}
            xb_t: dict = {}
            h1_t: dict = {}
            h2_t: dict = {}
            h3_t: dict = {}

            def emit_load(pk):
                # Per-pair quarter tiles: each pair's matmuls wait only on
                # their own 2*TB columns; xb quarter follows its xa quarter.
                c0 = pk * W
                qw = 2 * TB
                for qf in range(4):
                    ch = slice(c0 + qf * qw, c0 + (qf + 1) * qw)
                    xa = xa_pool.tile(
                        [K1A, qw], dt.bfloat16, tag="xa", name=f"xa_{pk}_{qf}"
                    )
                    nc.sync.dma_start(xa[:], xT[0:K1A, ch])
                    xb = xb_pool.tile(
                        [K1B, qw], dt.bfloat16, tag="xb", name=f"xb_{pk}_{qf}"
                    )
                    nc.sync.dma_start(xb[:], xT[K1A:D_IN, ch])
                    xa_t[(pk, qf)], xb_t[(pk, qf)] = xa, xb

            def stage_A(i):  # L1 for pair i: W1a(t0),W1a(t1),W1b(t0),W1b(t1)
                pk, pr = divmod(i, PACK // 2)
                xa, xb = xa_t[(pk, pr)], xb_t[(pk, pr)]
                ps = []
                for q in range(2):
                    t = 2 * i + q
                    o = q * TB
                    p1 = ps1.tile([H, TB], dt.float32, tag="p1", name=f"p1_{t}")
                    mm = nc.tensor.matmul(
                        p1[:], w1a_sb[:], xa[:, o : o + TB], start=True, stop=False
                    )
                    if q == 1 and noload_opt:
                        mm.ins.ldweights = False
                        noload.append(mm.ins)
                    ps.append((t, o, p1))
                for qq, (t, o, p1) in enumerate(ps):
                    mm = nc.tensor.matmul(
                        p1[:], w1b_sb[:], xb[:, o : o + TB], start=False, stop=True
                    )
                    if qq == 1 and noload_opt:
                        mm.ins.ldweights = False
                        noload.append(mm.ins)
                for t, o, p1 in ps:
                    h1 = h1_pool.tile([H, TB], dt.bfloat16, tag="h1", name=f"h1_{t}")
                    relu_evac(t % 2 == 0, h1, p1, b_sb[0])
                    h1_t[t] = h1

            def stage_B(i):  # L2 for pair i
                for q in range(2):
                    t = 2 * i + q
                    h1 = h1_t.pop(t)
                    p2 = ps2.tile([H, TB], dt.float32, tag="p2", name=f"p2_{t}")
                    mm = nc.tensor.matmul(p2[:], w2_sb[:], h1[:], start=True, stop=True)
                    if q == 1 and noload_opt:
                        mm.ins.ldweights = False
                        noload.append(mm.ins)
                    h2 = h2_pool.tile([H, TB], dt.bfloat16, tag="h2", name=f"h2_{t}")
                    relu_evac(t % 2 == 1, h2, p2, b_sb[1])
                    h2_t[t] = h2

            def stage_C(i):  # L3 for pair i
                for q in range(2):
                    t = 2 * i + q
                    h2 = h2_t.pop(t)
                    p3 = ps3.tile([H, TB], dt.float32, tag="p3", name=f"p3_{t}")
                    mm = nc.tensor.matmul(p3[:], w3_sb[:], h2[:], start=True, stop=True)
                    if q == 1 and noload_opt:
                        mm.ins.ldweights = False
                        noload.append(mm.ins)
                    h3 = h3_pool.tile([H, TB], dt.bfloat16, tag="h3", name=f"h3_{t}")
                    e3 = relu_evac(t % 2 == 0, h3, p3, b_sb[2])
                    h3_t[t] = (h3, e3)

            p4_t: dict = {}
            ysb_t: dict = {}

            def stage_Hj(pk, j):
                # head burst: variant j covers tiles 4j+u (u=0..3) of the
                # pack, i.e. pairs 4pk+2j, 4pk+2j+1 -- both freshly evacuated,
                # so the 4 matmuls are simultaneously ready and stay
                # contiguous -> concurrent via 4x column tiling.
                if j == 0:
                    p4_t[pk] = ps4.tile([H, TB], dt.float32, tag="p4", name=f"p4_{pk}")
                p4 = p4_t[pk]
                tiles = [8 * pk + 4 * j + u for u in range(4)]
                hs = [h3_t[t] for t in tiles]
                # Explicit per-col-group weight loads, then non-self-loading
                # matmuls: interleaved LDW+MM would serialize (a col-group LDW
                # conflicts with every in-flight MM on row groups). All 8
                # instructions become ready at the same event (both L3 evacs
                # done), so the burst schedules contiguously and nothing can
                # clobber the loaded weights in between.
                ldws = []
                for u in range(4):
                    ldw = nc.tensor.ldweights(
                        w4_sb[j][:], tile_position=(0, 32 * u)
                    )
                    for _, e3 in hs:
                        _add_dep_helper(ldw.ins, e3.ins, True, "head ldw after e3")
                    ldws.append(ldw)
                for u in range(4):
                    mm = nc.tensor.matmul(
                        p4[32 * u : 32 * u + 32, :],
                        w4_sb[j][:],
                        hs[u][0][:],
                        start=(j == 0),
                        stop=(j == 1),
                        tile_position=(0, 32 * u),
                        skip_group_check=True,
                    )
                    mm.ins.ldweights = False
                    _add_dep_helper(mm.ins, ldws[u].ins, False, "head mm after ldw")
                for t in tiles:
                    h3_t.pop(t)
                if j == 1:
                    p4 = p4_t.pop(pk)
                    ysb = y_pool.tile([H, TB], dt.float32, tag="ysb", name=f"ysb_{pk}")
                    nc.scalar.copy(ysb[:], p4[:])
                    # per-pack stores on GpSimd (SWDGE): small bursts that
                    # don't starve the next pack's loads, and their waits
                    # never block load triggers on the Sync sequencer
                    nc.gpsimd.dma_start(yTS[:, pk * TB : (pk + 1) * TB], ysb[:])

            # --- software-pipelined emission ---
            emit_load(0)
            for step in range(n_pairs + 11):
                # staggered prefetch: packs 1,2 during steps 0,1, then
                # pack (step+10)//4 at steps 2, 6, 10, ...
                if step in (0, 1) and step + 1 < n_packs:
                    emit_load(step + 1)
                if step % 4 == 0 and (step + 12) // 4 < n_packs:
                    emit_load((step + 12) // 4)
                ib = step - 2
                if 0 <= ib < n_pairs:
                    stage_B(ib)
                ic = step - 4
                if 0 <= ic < n_pairs:
                    stage_C(ic)
                    if ic % 4 == 1:
                        stage_Hj(ic // 4, 0)
                    elif ic % 4 == 3:
                        stage_Hj(ic // 4, 1)
                if step < n_pairs:
                    stage_A(step)

    nc.compile()
    if noload_opt:
        try:
            _verify_noload_safety(nc, noload)
        except AssertionError:
            # schedule changed in a way that makes weight reuse unsafe;
            # rebuild without the optimization (correctness first)
            return build_nc(b_core, n_cores, noload_opt=False)
    return nc


def _weights_key(inst, idx):
    ap = inst.ins[idx]
    s = str(ap)
    return s


def _verify_noload_safety(nc, noload):
    """The schedule is static: verify no other weight load lands between a
    ldweights=False matmul and the instruction that loaded its weights."""
    import concourse.mybir as mybir

    noload_ids = {id(i) for i in noload}
    cur = None  # weights key currently in the PE array (full-array loads)
    checked = 0
    insts = []
    for blk in nc.m.functions[0].blocks:
        insts.extend(blk.instructions)
    for inst in insts:
        if inst.engine != mybir.EngineType.PE:
            continue
        kind = type(inst).__name__
        if kind == "InstLdweights":
            tp = getattr(inst, "tile_position", None)
            if not tp or tuple(tp) == (0, 0):
                cur = _weights_key(inst, 0)
            else:
                cur = ("coltile", None)  # partial col-group load
        elif kind == "InstMatmult":
            if id(inst) in noload_ids:
                want = _weights_key(inst, 1)
                assert cur == want, (
                    f"noload matmul {inst.name} expects weights {want}, array has {cur}"
                )
                checked += 1
            elif getattr(inst, "ldweights", None) is False:
                pass  # head matmul: guarded by its own explicit ldw deps
            else:
                tp = getattr(inst, "tile_position", None)
                if not tp or tuple(tp) == (0, 0):
                    cur = _weights_key(inst, 1)
                else:
                    cur = ("coltile", None)
    assert checked == len(noload), (checked, len(noload))


def _prep_core_inputs(x_shard: np.ndarray, weights: dict) -> dict:
    xT = np.ascontiguousarray(x_shard.T).astype(BF16)
    return {"xT": xT, **weights}


def _prep_weights(W1, b1, W2, b2, W3, b3, W4) -> dict:
    w4a = np.zeros((32, H), np.float32)
    w4a[0:D_OUT] = W4
    w4b = np.zeros((32, H), np.float32)
    w4b[D_OUT : 2 * D_OUT] = W4
    return {
        "w1t": np.ascontiguousarray(np.sign(W1).T).astype(BF16),
        "w2t": np.ascontiguousarray(np.sign(W2).T).astype(BF16),
        "w3t": np.ascontiguousarray(np.sign(W3).T).astype(BF16),
        "w4a": np.ascontiguousarray(w4a.T).astype(BF16),
        "w4b": np.ascontiguousarray(w4b.T).astype(BF16),
        "b1": b1.reshape(H, 1).astype(np.float32),
        "b2": b2.reshape(H, 1).astype(np.float32),
        "b3": b3.reshape(H, 1).astype(np.float32),
    }


def _unscramble(yTS: np.ndarray, b_core: int) -> np.ndarray:
    """yTS [128, n_packs*TB] strip layout -> y_core [b_core, 10]."""
    n_packs = b_core // (PACK * TB)
    # yTS[32u+10j+p, pk*TB+c] = y[(pk*8+4j+u)*TB + c, p]
    v = yTS.reshape(4, 32, n_packs, TB)[:, :20]  # [u, 10j+p, pk, c]
    v = v.reshape(4, 2, 10, n_packs, TB)  # [u, j, p, pk, c]
    # -> y[pk, j, u, c, p]
    y = v.transpose(3, 1, 0, 4, 2).reshape(b_core, D_OUT)
    return y


_NC_CACHE: dict = {}


def run(x, W1, b1, W2, b2, W3, b3, W4, b4, trace=False, trace_kwargs=None):
    """Run the SPMD kernel on 8 cores; returns (y, BassKernelResults)."""
    x = np.asarray(x, dtype=np.float32)
    b_total = x.shape[0]
    assert b_total % N_CORES == 0
    b_core = b_total // N_CORES

    key = b_core
    if key not in _NC_CACHE:
        _NC_CACHE[key] = build_nc(b_core)
    nc = _NC_CACHE[key]

    weights = _prep_weights(
        np.asarray(W1), np.asarray(b1), np.asarray(W2), np.asarray(b2),
        np.asarray(W3), np.asarray(b3), np.asarray(W4),
    )
    in_maps = [
        _prep_core_inputs(x[c * b_core : (c + 1) * b_core], weights)
        for c in range(N_CORES)
    ]
    res = run_bass_kernel_spmd(
        nc,
        in_maps,
        list(range(N_CORES)),
        trace=trace,
        **(trace_kwargs or {}),
    )
    b4f = np.asarray(b4, dtype=np.float32)
    y = np.empty((b_total, D_OUT), dtype=np.float32)
    for c in range(N_CORES):
        y[c * b_core : (c + 1) * b_core] = _unscramble(res.results[c]["yTS"], b_core)
    y += b4f
    return y, res


def kernel(x, W1, b1, W2, b2, W3, b3, W4, b4):
    y, _ = run(x, W1, b1, W2, b2, W3, b3, W4, b4)
    return y
